# revision 14
# baseline (speedup 1.0000x reference)
"""Self-contained Trainium2 Bass kernel for nn_CrossStageAttention.

Data-parallel over batch: 16 images -> 8 NeuronCores x 2 images each.
Training-mode BatchNorm statistics are made global via two tiny AllReduces.

v2: bf16 on-chip everywhere (f32 PSUM accumulation), all transposes moved to
the DMA xbar (dma_start_transpose, 16x128 tiles), no DRAM spills (fusx/y/xT
stay SBUF-resident), batched DMA loads/stores, 2x2 avg-pool sum kept
unnormalized (the 1/4 is folded into the softmax scale and the beta weight).

The torch "(attn@v).transpose(1,2).reshape" scramble is absorbed into the
fuse access patterns exactly as in the f32r baseline.
"""
import numpy as np
import ml_dtypes
from contextlib import ExitStack

import concourse.bass as bass
import concourse.tile as tile
import concourse.bacc as bacc
from concourse import mybir
from concourse.bass_utils import run_bass_kernel_spmd

N_CORES = 8
IMGS = 2
C = 512
N = 1024          # query positions per image (32x32)
PC = 256
MP = 4096         # prev positions per image (64x64)
F32 = mybir.dt.float32
BF = mybir.dt.bfloat16
BF_NP = ml_dtypes.bfloat16
SCALE = 32 ** -0.5
B0_SELF = 128.0   # constant softmax-stabilization bias for self-attention
EPS = 1e-5
INV_CNT = 1.0 / (16 * 1024)
AF = mybir.ActivationFunctionType
ALU = mybir.AluOpType
X_AXIS = mybir.AxisListType.X


def build_nc():
    nc = bacc.Bacc("TRN2", target_bir_lowering=False, debug=False,
                   num_devices=N_CORES)
    x_d = nc.dram_tensor("x", [IMGS, N, C], BF, kind="ExternalInput").ap()
    px_d = nc.dram_tensor("px", [IMGS, MP, PC], BF, kind="ExternalInput").ap()
    wq_d = nc.dram_tensor("wq", [C, C], BF, kind="ExternalInput").ap()
    wp_d = nc.dram_tensor("wp", [PC, C], BF, kind="ExternalInput").ap()
    fw_d = nc.dram_tensor("fw", [2 * C, C], BF, kind="ExternalInput").ap()
    ow_d = nc.dram_tensor("ow", [9, C, C], BF, kind="ExternalInput").ap()
    g1_d = nc.dram_tensor("g1", [128, 4], F32, kind="ExternalInput").ap()
    b1_d = nc.dram_tensor("b1", [128, 4], F32, kind="ExternalInput").ap()
    g2_d = nc.dram_tensor("g2", [1, C], F32, kind="ExternalInput").ap()
    b2_d = nc.dram_tensor("b2", [1, C], F32, kind="ExternalInput").ap()
    pars_d = nc.dram_tensor("pars", [1, 2], F32, kind="ExternalInput").ap()
    out_d = nc.dram_tensor("out", [IMGS, N, C], F32, kind="ExternalOutput").ap()

    with tile.TileContext(nc) as tc, ExitStack() as ctx:
        const = ctx.enter_context(tc.tile_pool(name="const", bufs=1))
        wp_pool = ctx.enter_context(tc.tile_pool(name="wpool", bufs=1))
        pers = ctx.enter_context(tc.tile_pool(name="pers", bufs=1))
        scr = ctx.enter_context(tc.tile_pool(name="scr", bufs=10))
        ea_pool = ctx.enter_context(tc.tile_pool(name="eap", bufs=9))
        sm = ctx.enter_context(tc.tile_pool(name="sm", bufs=6))
        ps = ctx.enter_context(tc.tile_pool(name="ps", bufs=8, space="PSUM"))
        dram = ctx.enter_context(tc.tile_pool(name="dram", bufs=1, space="DRAM"))

        # ------------- DRAM scratch (collectives only) -------------
        bn1_in = dram.tile([128, 8], F32, tag="bn1i")
        bn1_out = dram.tile([128, 8], F32, tag="bn1o")
        bn2_in = dram.tile([1, 1024], F32, tag="bn2i")
        bn2_out = dram.tile([1, 1024], F32, tag="bn2o")

        # ------------- constants / params -------------
        onesF = const.tile([128, 2], F32, tag="onesF")
        nc.gpsimd.memset(onesF[:], 1.0)
        ones2 = const.tile([128, 2], BF, tag="ones2")
        nc.vector.tensor_copy(ones2[:], onesF[:])
        b0s = const.tile([128, 1], F32, tag="b0s")
        nc.gpsimd.memset(b0s[:], -B0_SELF)
        eps_t = const.tile([128, 1], F32, tag="eps")
        nc.gpsimd.memset(eps_t[:], EPS)
        g1_s = const.tile([128, 4], F32, tag="g1")
        b1_s = const.tile([128, 4], F32, tag="b1")
        pars_s = const.tile([1, 2], F32, tag="pars")
        pars_bc = const.tile([128, 2], F32, tag="parsbc")
        s1acc = const.tile([128, 4, 4], F32, tag="s1acc")
        ss1acc = const.tile([128, 4, 4], F32, tag="ss1acc")
        s1v = const.tile([128, 4], F32, tag="s1v")
        t1v = const.tile([128, 4], F32, tag="t1v")
        nc.sync.dma_start(g1_s[:], g1_d)
        nc.sync.dma_start(b1_s[:], b1_d)
        nc.sync.dma_start(pars_s[:], pars_d)
        nc.gpsimd.partition_broadcast(pars_bc[:], pars_s[:])

        # ------------- weights (bf16, batched loads) -------------
        wq_s = wp_pool.tile([128, 4, C], BF, tag="wq")
        wp_s = wp_pool.tile([128, 2, C], BF, tag="wp")
        fw_s = wp_pool.tile([128, 8, C], BF, tag="fw")
        ow_s = wp_pool.tile([128, 9, 4, C], BF, tag="ow")
        nc.sync.dma_start(wq_s[:], wq_d.rearrange("(ic p) c -> p ic c", p=128))
        nc.sync.dma_start(wp_s[:], wp_d.rearrange("(ic p) c -> p ic c", p=128))
        nc.sync.dma_start(fw_s[:], fw_d.rearrange("(ic p) o -> p ic o", p=128))
        nc.sync.dma_start(ow_s[:],
                          ow_d.rearrange("t (ic p) o -> p t ic o", p=128))

        # ------------- persistent bf16 activations -------------
        fusx_t = pers.tile([128, IMGS, 4, 2, 512], BF, tag="fusx")
        y_t = pers.tile([128, IMGS, 8, 512], BF, tag="y")
        xTk_t = pers.tile([128, IMGS, 4, N], BF, tag="xTk")

        # =================== attention scope ===================
        with tc.tile_pool(name="attn", bufs=1) as ap_:
            for img in range(IMGS):
                qT_t = ap_.tile([128, 4, N], BF, tag="qT", name="qT")
                xnow_t = ap_.tile([128, 8, C], BF, tag="xnow", name="xnow")
                xprev_t = ap_.tile([128, 8, C], BF, tag="xprev", name="xprev")

                def do_attn(kind, kvT, vnat):
                    bias = b0s[:] if kind == "self" else 0.0
                    scale = SCALE * 0.25 if kind == "avg" else SCALE
                    for nh in range(2):
                        eas = []
                        for mi in range(8):
                            lg = ps.tile([128, 512], F32, tag="b", name="lg")
                            for ci in range(4):
                                nc.tensor.matmul(
                                    lg[:],
                                    kvT[:, ci, 128 * mi:128 * mi + 128],
                                    qT_t[:, ci, 512 * nh:512 * nh + 512],
                                    start=(ci == 0), stop=(ci == 3))
                            ea = ea_pool.tile([128, 512], BF, tag="ea",
                                              name="ea")
                            nc.scalar.activation(ea[:], lg[:], AF.Exp,
                                                 bias=bias, scale=scale)
                            eas.append(ea)
                        for np2 in range(2):
                            o_ps = [ps.tile([128, 512], F32, tag="b",
                                            name="ops") for _ in range(2)]
                            s_ps = [ps.tile([128, 512], F32, tag="b",
                                            name="sps") for _ in range(2)]
                            for mi in range(8):
                                for k in range(2):
                                    lhsT = eas[mi][:, 128 * (2 * np2 + k):
                                                   128 * (2 * np2 + k) + 128]
                                    nc.tensor.matmul(o_ps[k][:], lhsT,
                                                     vnat[:, mi, :],
                                                     start=(mi == 0),
                                                     stop=(mi == 7))
                                    nc.tensor.matmul(s_ps[k][:, 0:2], lhsT,
                                                     ones2[:],
                                                     start=(mi == 0),
                                                     stop=(mi == 7))
                            for k in range(2):
                                nck = 4 * nh + 2 * np2 + k
                                rec = sm.tile([128, 4], F32, name="rec")
                                nc.vector.reciprocal(rec[:, 0:1],
                                                     s_ps[k][:, 0:1])
                                if kind == "self":
                                    nc.vector.tensor_scalar_mul(
                                        xnow_t[:, nck, :], o_ps[k][:],
                                        rec[:, 0:1])
                                elif kind == "avg":
                                    w = sm.tile([128, 4], F32, name="bw")
                                    nc.vector.tensor_tensor(
                                        w[:, 0:1], rec[:, 0:1],
                                        pars_bc[:, 0:1], op=ALU.mult)
                                    nc.vector.tensor_scalar_mul(
                                        xprev_t[:, nck, :], o_ps[k][:],
                                        w[:, 0:1])
                                else:
                                    w = sm.tile([128, 4], F32, name="bw")
                                    nc.vector.tensor_tensor(
                                        w[:, 0:1], rec[:, 0:1],
                                        pars_bc[:, 1:2], op=ALU.mult)
                                    nc.vector.scalar_tensor_tensor(
                                        xprev_t[:, nck, :], o_ps[k][:],
                                        w[:, 0:1], xprev_t[:, nck, :],
                                        op0=ALU.mult, op1=ALU.add)

                # ---- x^T via DMA xbar transpose (per 128-ch chunk)
                xT = xTk_t[:, img]
                for ci in range(4):
                    nc.sync.dma_start_transpose(
                        xTk_t[:, img, ci], x_d[img, :, 128 * ci:128 * ci + 128])
                # ---- qkv^T projection
                for ci in range(4):
                    for nh in range(2):
                        qp = ps.tile([128, 512], F32, tag="b", name="qp")
                        for ic in range(4):
                            nc.tensor.matmul(
                                qp[:], wq_s[:, ic, 128 * ci:128 * ci + 128],
                                xT[:, ic, 512 * nh:512 * nh + 512],
                                start=(ic == 0), stop=(ic == 3))
                        nc.scalar.copy(qT_t[:, ci, 512 * nh:512 * nh + 512],
                                       qp[:])
                # ---- qkv natural via DMA transpose
                nat_t = ap_.tile([128, 8, C], BF, tag="nat", bufs=2,
                                 name="qkvnat")
                for ci in range(4):
                    nc.sync.dma_start_transpose(
                        nat_t[:, :, 128 * ci:128 * ci + 128], qT_t[:, ci, :])
                do_attn("self", qT_t, nat_t)

                # ---- prevx^T (1 instr) -> prevqkv^T chunks + 2x2 pooling
                pxT_t = ap_.tile([128, 2, MP], BF, tag="pxT", name="pxT")
                for pc in range(2):
                    nc.sync.dma_start_transpose(
                        pxT_t[:, pc, :], px_d[img, :, 128 * pc:128 * pc + 128])
                avgT_t = ap_.tile([128, 4, N], BF, tag="avgT", name="avgT")
                maxT_t = ap_.tile([128, 4, N], BF, tag="maxT", name="maxT")
                for ch in range(8):
                    for ci in range(4):
                        pq = ps.tile([128, 512], F32, tag="b", name="pq")
                        for pc in range(2):
                            nc.tensor.matmul(
                                pq[:], wp_s[:, pc, 128 * ci:128 * ci + 128],
                                pxT_t[:, pc, 512 * ch:512 * ch + 512],
                                start=(pc == 0), stop=(pc == 1))
                        # PSUM -> SBUF bf16 (Act), then pool from SBUF
                        pqs = scr.tile([128, 512], BF, tag="pqs", bufs=3,
                                       name="pqs")
                        nc.scalar.copy(pqs[:], pq[:])
                        v = pqs[:].rearrange("p (i a j b) -> p i a j b",
                                             i=4, a=2, j=32, b=2)
                        pm = scr.tile([128, 2, 4, 2, 32], BF, tag="pool",
                                      bufs=3, name="pm")
                        nc.vector.tensor_tensor(pm[:, 0], v[:, :, :, :, 0],
                                                v[:, :, :, :, 1], op=ALU.max)
                        nc.vector.tensor_tensor(pm[:, 1], v[:, :, :, :, 0],
                                                v[:, :, :, :, 1], op=ALU.add)
                        # stage 2 (pair over h) on Pool engine, SBUF bf16
                        nc.vector.tensor_tensor(
                            maxT_t[:, ci, 128 * ch:128 * ch + 128]
                            .rearrange("p (i j) -> p i j", j=32),
                            pm[:, 0, :, 0, :], pm[:, 0, :, 1, :], op=ALU.max)
                        nc.vector.tensor_tensor(
                            avgT_t[:, ci, 128 * ch:128 * ch + 128]
                            .rearrange("p (i j) -> p i j", j=32),
                            pm[:, 1, :, 0, :], pm[:, 1, :, 1, :], op=ALU.add)
                # ---- avg attention (unnormalized sum; 1/4 folded in scale
                #      and in host-side beta)
                nat_t = ap_.tile([128, 8, C], BF, tag="nat", bufs=2,
                                 name="avgnat")
                for ci in range(4):
                    nc.sync.dma_start_transpose(
                        nat_t[:, :, 128 * ci:128 * ci + 128], avgT_t[:, ci, :])
                do_attn("avg", avgT_t, nat_t)
                # ---- max attention
                nat_t = ap_.tile([128, 8, C], BF, tag="nat", bufs=2,
                                 name="maxnat")
                for ci in range(4):
                    nc.sync.dma_start_transpose(
                        nat_t[:, :, 128 * ci:128 * ci + 128], maxT_t[:, ci, :])
                do_attn("max", maxT_t, nat_t)

                # ---- fuse matmul + BN1 partial stats, fusx SBUF-resident
                for oi in range(4):
                    for v2 in range(2):
                        fp = ps.tile([128, 512], F32, tag="b", name="fp")
                        for ii in range(8):
                            rhs = (xnow_t[:, 4 * v2 + ii, :] if ii < 4
                                   else xprev_t[:, 4 * v2 + (ii - 4), :])
                            nc.tensor.matmul(
                                fp[:], fw_s[:, ii, 128 * oi:128 * oi + 128],
                                rhs, start=(ii == 0), stop=(ii == 7))
                        slot = 2 * img + v2
                        nc.scalar.copy(fusx_t[:, img, oi, v2, :], fp[:])
                        nc.vector.tensor_reduce(
                            s1acc[:, oi, slot:slot + 1], fp[:],
                            axis=X_AXIS, op=ALU.add)
                        fsq = scr.tile([128, 512], F32, tag="sq", bufs=2, name="fsq")
                        nc.scalar.square(fsq[:], fp[:])
                        nc.vector.tensor_reduce(
                            ss1acc[:, oi, slot:slot + 1], fsq[:],
                            axis=X_AXIS, op=ALU.add)

        # =================== BN1 global stats ===================
        sum1 = sm.tile([128, 4], F32, name="sum1")
        ssq1 = sm.tile([128, 4], F32, name="ssq1")
        nc.vector.tensor_reduce(sum1[:], s1acc[:], axis=X_AXIS, op=ALU.add)
        nc.vector.tensor_reduce(ssq1[:], ss1acc[:], axis=X_AXIS, op=ALU.add)
        nc.gpsimd.dma_start(bn1_in[:, 0:4], sum1[:])
        nc.gpsimd.dma_start(bn1_in[:, 4:8], ssq1[:])
        nc.gpsimd.collective_compute(
            "AllReduce", ALU.add, replica_groups=[list(range(N_CORES))],
            ins=[bn1_in.opt()], outs=[bn1_out.opt()])
        allst = sm.tile([128, 8], F32, name="allst")
        nc.sync.dma_start(allst[:], bn1_out[:])
        mean1 = sm.tile([128, 4], F32, name="mean1")
        tA = sm.tile([128, 4], F32, name="tA")
        tB = sm.tile([128, 4], F32, name="tB")
        nc.scalar.mul(mean1[:], allst[:, 0:4], INV_CNT)
        nc.scalar.mul(tA[:], allst[:, 4:8], INV_CNT)
        nc.scalar.square(tB[:], mean1[:])
        nc.vector.tensor_tensor(tA[:], tA[:], tB[:], op=ALU.subtract)
        nc.scalar.activation(tA[:], tA[:], AF.Sqrt, bias=eps_t[:])
        nc.vector.reciprocal(tA[:], tA[:])
        nc.vector.tensor_tensor(s1v[:], g1_s[:], tA[:], op=ALU.mult)
        nc.vector.tensor_tensor(tB[:], mean1[:], s1v[:], op=ALU.mult)
        nc.vector.tensor_tensor(t1v[:], b1_s[:], tB[:], op=ALU.subtract)

        # =================== conv scope ===================
        with tc.tile_pool(name="conv", bufs=1) as cp_, \
                tc.tile_pool(name="rows", bufs=1) as rows:
            # ---- BN1 apply + residual -> 3 shifted vert-padded x2 buffers
            # x2s[d][(h+1)*32 + w - d + 1] = x2[h, w]   (d = kernel dw)
            sy_ps = ps.tile([128, 512], F32, tag="b", name="syp")
            sy2_ps = ps.tile([128, 512], F32, tag="b", name="sy2p")
            first = True
            for img in range(IMGS):
                x2s = cp_.tile([128, 3, 4, 1088], BF, tag="x2s", bufs=2,
                               name=f"x2s{img}")
                for d in range(3):
                    for ci in range(4):
                        x2v = (x2s[:, d, ci]
                               .rearrange("p (ph pw) -> p ph pw", pw=32))
                        nc.gpsimd.memset(x2v[:, 0, :], 0.0)
                        nc.gpsimd.memset(x2v[:, 33, :], 0.0)
                        if d == 0:
                            nc.gpsimd.memset(x2v[:, 1:33, 0], 0.0)
                        if d == 2:
                            nc.gpsimd.memset(x2v[:, 1:33, 31], 0.0)
                for oi in range(4):
                    xr = xTk_t[:, img, oi]
                    for v2 in range(2):
                        rt = scr.tile([128, 512], BF, tag="rt", bufs=4, name="rt")
                        nc.scalar.activation(rt[:], fusx_t[:, img, oi, v2, :],
                                             AF.Relu,
                                             bias=t1v[:, oi:oi + 1],
                                             scale=s1v[:, oi:oi + 1])
                        rtv = rt[:].rearrange("p (a b) -> p a b", b=16)
                        xin = (xr.rearrange("p (a b two) -> p a b two",
                                            a=32, two=2)[:, :, :, v2])
                        for d in range(3):
                            d0 = v2 - d + 1
                            pad = (x2s[:, d, oi]
                                   .rearrange("p (ph pw) -> p ph pw", pw=32)
                                   [:, 1:33, :]
                                   .rearrange("p a (b2 two) -> p a b2 two",
                                              two=2))
                            if d0 == 2:
                                dst, sl = pad[:, :, 1:16, 0], slice(0, 15)
                            elif d0 == 1:
                                dst, sl = pad[:, :, 0:16, 1], slice(0, 16)
                            elif d0 == 0:
                                dst, sl = pad[:, :, 0:16, 0], slice(0, 16)
                            else:  # d0 == -1
                                dst, sl = pad[:, :, 0:15, 1], slice(1, 16)
                            nc.vector.tensor_tensor(
                                dst, rtv[:, :, sl], xin[:, :, sl], op=ALU.add)

                # ---- conv 3x3 + BN2 stats (y SBUF-resident)
                for t in range(8):
                    yp = ps.tile([128, 512], F32, tag="b", name="yp")
                    k = 0
                    for tap in range(9):
                        dh, dw = tap // 3, tap % 3
                        for ii in range(4):
                            lhsT = x2s[:, dw, ii,
                                       32 * (4 * t + dh):
                                       32 * (4 * t + dh) + 128]
                            nc.tensor.matmul(yp[:], lhsT, ow_s[:, tap, ii, :],
                                             start=(k == 0), stop=(k == 35))
                            k += 1
                    nc.scalar.copy(y_t[:, img, t, :], yp[:])
                    ysq = scr.tile([128, 512], BF, tag="ysq", bufs=3, name="ysq")
                    nc.scalar.square(ysq[:], yp[:])
                    last = (img == IMGS - 1 and t == 7)
                    nc.tensor.matmul(sy_ps[0:1, :], ones2[:, 0:1],
                                     y_t[:, img, t, :],
                                     start=first, stop=last)
                    nc.tensor.matmul(sy2_ps[0:1, :], ones2[:, 0:1], ysq[:],
                                     start=first, stop=last)
                    first = False

            # ---- BN2 global stats
            syr = rows.tile([1, 512], F32, tag="syr", name="syr")
            sy2r = rows.tile([1, 512], F32, tag="sy2r", name="sy2r")
            nc.vector.tensor_copy(syr[:], sy_ps[0:1, :])
            nc.vector.tensor_copy(sy2r[:], sy2_ps[0:1, :])
            nc.gpsimd.dma_start(bn2_in[:, 0:512], syr[:])
            nc.gpsimd.dma_start(bn2_in[:, 512:1024], sy2r[:])
            nc.gpsimd.collective_compute(
                "AllReduce", ALU.add, replica_groups=[list(range(N_CORES))],
                ins=[bn2_in.opt()], outs=[bn2_out.opt()])
            st2 = rows.tile([1, 1024], F32, tag="st2", bufs=1, name="st2")
            nc.sync.dma_start(st2[:], bn2_out[:])
            g2_s = rows.tile([1, C], F32, tag="g2s", name="g2s")
            b2_s = rows.tile([1, C], F32, tag="b2s", name="b2s")
            nc.sync.dma_start(g2_s[:], g2_d)
            nc.sync.dma_start(b2_s[:], b2_d)
            mean2 = rows.tile([1, 512], F32, tag="mean2", name="mean2")
            s2v = rows.tile([1, 512], F32, tag="s2v", name="s2v")
            t2v = rows.tile([1, 512], F32, tag="t2v", name="t2v")
            u1 = rows.tile([1, 512], F32, tag="u1", name="u1")
            u2 = rows.tile([1, 512], F32, tag="u2", name="u2")
            nc.scalar.mul(mean2[:], st2[:, 0:512], INV_CNT)
            nc.scalar.mul(u1[:], st2[:, 512:1024], INV_CNT)
            nc.scalar.square(u2[:], mean2[:])
            nc.vector.tensor_tensor(u1[:], u1[:], u2[:], op=ALU.subtract)
            nc.scalar.activation(u1[:], u1[:], AF.Sqrt, bias=eps_t[0:1, :])
            nc.vector.reciprocal(u1[:], u1[:])
            nc.vector.tensor_tensor(s2v[:], g2_s[:], u1[:], op=ALU.mult)
            nc.vector.tensor_tensor(u2[:], mean2[:], s2v[:], op=ALU.mult)
            nc.vector.tensor_tensor(t2v[:], b2_s[:], u2[:], op=ALU.subtract)
            s2vb = rows.tile([1, C], BF, tag="s2vb", name="s2vb")
            t2vb = rows.tile([1, C], BF, tag="t2vb", name="t2vb")
            nc.vector.tensor_copy(s2vb[:], s2v[:])
            nc.vector.tensor_copy(t2vb[:], t2v[:])
            s2bc = cp_.tile([128, 512], BF, tag="s2bc", name="s2bc")
            t2bc = cp_.tile([128, 512], BF, tag="t2bc", name="t2bc")
            nc.gpsimd.partition_broadcast(s2bc[:], s2vb[:])
            nc.gpsimd.partition_broadcast(t2bc[:], t2vb[:])

            # ---- BN2 apply + store (split DVE / Pool, batched stores)
            for img in range(IMGS):
                for t in range(8):
                    eng = nc.vector
                    w1 = scr.tile([128, 512], BF, tag="rt", bufs=4, name="w1s")
                    eng.tensor_tensor(w1[:], y_t[:, img, t, :], s2bc[:],
                                      op=ALU.mult)
                    eng.tensor_tensor(w1[:], w1[:], t2bc[:], op=ALU.add)
                    w1f = scr.tile([128, 512], F32, tag="w1f", bufs=2,
                                   name="w1f")
                    eng.tensor_scalar_max(w1f[:], w1[:], 0.0)
                    nc.sync.dma_start(
                        out_d[img, 128 * t:128 * t + 128, :], w1f[:])

    nc.compile()
    return nc


_STATE = {}


def _get_nc():
    if "nc" not in _STATE:
        _STATE["nc"] = build_nc()
    return _STATE["nc"]


def make_in_maps(x, prevx, w_prev_qkv, w_qkv, fuse_w, fuse_b, bn1_g, bn1_b,
                 out_w, out_b, bn2_g, bn2_b, gamma, beta):
    f = np.float32
    wq = np.ascontiguousarray(np.asarray(w_qkv, f).T).astype(BF_NP)
    wp = np.ascontiguousarray(np.asarray(w_prev_qkv, f).T).astype(BF_NP)
    fw = np.ascontiguousarray(np.asarray(fuse_w, f)).astype(BF_NP)
    ow = np.ascontiguousarray(
        np.asarray(out_w, f).reshape(9, C, C)).astype(BF_NP)
    g = float(np.asarray(gamma, f).reshape(-1)[0])
    g1 = np.ascontiguousarray((g * np.asarray(bn1_g, f)).reshape(4, 128).T)
    b1 = np.ascontiguousarray((g * np.asarray(bn1_b, f)).reshape(4, 128).T)
    g2 = np.ascontiguousarray(np.asarray(bn2_g, f).reshape(1, C))
    b2 = np.ascontiguousarray(np.asarray(bn2_b, f).reshape(1, C))
    bt = float(np.asarray(beta, f).reshape(-1)[0])
    # avg-pool keeps the raw 2x2 sum: fold the 1/4 into beta (the exp scale
    # handles the K side)
    pars = np.array([[bt * 0.25, 1.0 - bt]], f)
    xf = np.asarray(x, f).reshape(16, N, C).astype(BF_NP)
    pxf = np.asarray(prevx, f).reshape(16, MP, PC).astype(BF_NP)
    maps = []
    for c in range(N_CORES):
        maps.append({
            "x": np.ascontiguousarray(xf[2 * c:2 * c + 2]),
            "px": np.ascontiguousarray(pxf[2 * c:2 * c + 2]),
            "wq": wq, "wp": wp, "fw": fw, "ow": ow,
            "g1": g1, "b1": b1, "g2": g2, "b2": b2, "pars": pars,
        })
    return maps


def kernel(**inputs):
    nc = _get_nc()
    maps = make_in_maps(**inputs)
    res = run_bass_kernel_spmd(nc, maps, list(range(N_CORES)))
    out = np.concatenate([res.results[c]["out"] for c in range(N_CORES)],
                         axis=0)
    return out.reshape(16, 32, 32, C).astype(np.float32)
